# revision 3
# baseline (speedup 1.0000x reference)
"""Trainium2 Bass kernel for nn_MixtureLinear.

Math:  out[b,n,d] = sum_{c,r} input[b,n,c] * weight[d,c,r] * coef[n,r]
                    + sum_r coef[n,r] * bias[d,r]

Sharding: data-parallel over batch (B == 8 == n_cores).

Per-core formulation: ONE fused matmul with contraction K = C*R = 8192 by
folding coef into the activations host-side:
    xp[r*C+c, n] = input[b,n,c] * coef[n,r]      (lhsT, resident in SBUF)
    wt[r*C+c, d] = weight[d,c,r]                 (rhs, streamed from HBM)
    out[n, d]    = xp.T @ wt + coef @ bias.T
Each [128n x 512d] output tile accumulates 64 matmuls in one PSUM bank with
no intermediate drains (vs 8 rank-passes + DVE scaling in v1). The bias term
for the first d-half rides as a K=8 matmul that doubles as PE warmup; the
second half is added by the DVE during the PSUM drain.

Startup: the tensor engine p-state ramps to full speed only after ~3us of
continuous work, so a run of tiny F=64 dummy matmuls (reading a memset SBUF
tile, no DMA dependency) starts the ramp at ~1.5us while the first real
tiles stream in.

DMA queues: sync = wt stream (both halves, in consumption order), gpsimd =
xp stream + output stores, scalar = biasnd + dt0 drains, vector = dt1
drains. Keeps every drain engine idle at the moment its drains are due.
"""

import sys

if "/opt/trn_rl_repo" not in sys.path:
    sys.path.insert(0, "/opt/trn_rl_repo")

import numpy as np

B, N, C, D, R = 8, 1024, 1024, 1024, 8
P = 128        # SBUF partitions
DTILE = 512    # matmul moving free dim (one fp32 PSUM bank)
K = C * R      # fused contraction
KT = K // P    # 64 contraction tiles
MT = N // P    # 8 token tiles
DT = D // DTILE  # 2 output column tiles
N_CORES = 8
NDUMMY = 16    # warmup matmuls (F=64) to ramp the PE p-state during DMA wait

_CACHE = {}


def _build_nc():
    import concourse.mybir as mybir
    import concourse.tile as tile
    from concourse import bacc

    f32 = mybir.dt.float32
    bf16 = mybir.dt.bfloat16
    mult = mybir.AluOpType.mult
    add = mybir.AluOpType.add

    nc = bacc.Bacc()
    xp = nc.dram_tensor("xp", [K, N], bf16, kind="ExternalInput")
    wt = nc.dram_tensor("wt", [K, D], bf16, kind="ExternalInput")
    coefT = nc.dram_tensor("coefT", [R, N], bf16, kind="ExternalInput")
    biasT = nc.dram_tensor("biasT", [R, D], bf16, kind="ExternalInput")
    biasnd1 = nc.dram_tensor("biasnd1", [N, DTILE], bf16, kind="ExternalInput")
    out = nc.dram_tensor("out", [N, D], f32, kind="ExternalOutput")

    with tile.TileContext(nc) as tc:
        with (
            tc.tile_pool(name="consts", bufs=1) as cpool,
            tc.tile_pool(name="wpool", bufs=22) as wpool,
            tc.tile_pool(name="stpool", bufs=4) as stpool,
            tc.tile_pool(name="psum", bufs=1, space="PSUM") as pspool,
        ):
            # ---- PSUM banks: one [128, 512] f32 accumulator per token tile,
            # reused across both d-halves (8 banks = whole PSUM).
            ps = [
                pspool.tile([P, DTILE], f32, name=f"ps{m}", tag=f"ps{m}", bufs=1)
                for m in range(MT)
            ]

            # ---- warmup: memset a small bf16 tile, then dummy matmuls that
            # keep the PE busy (ramping) while the first real DMAs land.
            warm = cpool.tile([P, 64], bf16, name="warm", tag="warm")
            nc.vector.memset(warm, 0.0)
            for _ in range(NDUMMY):
                nc.tensor.matmul(
                    ps[0][0:64, 0:64], warm, warm[:, 0:64], start=True, stop=True
                )

            # ---- tiny bias operands (sync queue, ahead of everything else)
            coefT_sb = cpool.tile([R, N], bf16, name="coefT_sb", tag="coefT_sb")
            nc.sync.dma_start(coefT_sb, coefT[:, :])
            biasT_sb = cpool.tile([R, D], bf16, name="biasT_sb", tag="biasT_sb")
            nc.sync.dma_start(biasT_sb, biasT[:, :])

            # first real tiles on sync: wt[0,0] then xp[0]
            wt_sb = {}
            wt_sb[0, 0] = wpool.tile([P, DTILE], bf16, name="w", tag="w")
            nc.sync.dma_start(wt_sb[0, 0], wt[0:P, 0:DTILE])

            xp_sb = [
                cpool.tile([P, N], bf16, name=f"xp{k}", tag=f"xp{k}")
                for k in range(KT)
            ]
            nc.sync.dma_start(xp_sb[0][:, 0:512], xp[0:P, 0:512])
            nc.sync.dma_start(xp_sb[0][:, 512:1024], xp[0:P, 512:1024])

            # remaining wt stream on sync, in consumption order (dt0 then dt1)
            for dt in range(DT):
                for k in range(KT):
                    if (k, dt) in wt_sb:
                        continue
                    w = wpool.tile([P, DTILE], bf16, name="w", tag="w")
                    nc.sync.dma_start(
                        w, wt[k * P : (k + 1) * P, dt * DTILE : (dt + 1) * DTILE]
                    )
                    wt_sb[k, dt] = w

            # xp stream on gpsimd (xp[0] already on sync)
            for k in range(1, KT):
                nc.gpsimd.dma_start(xp_sb[k], xp[k * P : (k + 1) * P, :])

            # biasnd for the second d-half on scalar (early; needed at ~220us)
            bias1_sb = []
            for m in range(MT):
                t = cpool.tile([P, DTILE], bf16, name=f"b1_{m}", tag=f"b1_{m}")
                nc.scalar.dma_start(t, biasnd1[m * P : (m + 1) * P, :])
                bias1_sb.append(t)

            # ---- dt0: bias term via K=8 matmul (start=True), then the 64-deep
            # fused accumulation; drain with scalar.copy (bias already in).
            # ---- dt1: start=True on k=0; drain adds biasnd via DVE stc.
            for dt in range(DT):
                dsl = slice(dt * DTILE, (dt + 1) * DTILE)
                if dt == 0:
                    for m in range(MT):
                        nc.tensor.matmul(
                            ps[m],
                            coefT_sb[:, m * P : (m + 1) * P],
                            biasT_sb[:, dsl],
                            start=True,
                            stop=False,
                        )
                for k in range(KT):
                    w = wt_sb.pop((k, dt))
                    for m in range(MT):
                        nc.tensor.matmul(
                            ps[m],
                            xp_sb[k][:, m * P : (m + 1) * P],
                            w,
                            start=(dt > 0 and k == 0),
                            stop=(k == KT - 1),
                        )
                for m in range(MT):
                    stage = stpool.tile([P, DTILE], f32, name="stage", tag="stage")
                    if dt == 0:
                        nc.scalar.copy(stage, ps[m])
                    else:
                        nc.vector.scalar_tensor_tensor(
                            stage, ps[m], 1.0, bias1_sb[m], mult, add
                        )
                    # output stores: mid-kernel rides gpsimd unsplit; the final
                    # d-half gets graded splits across idle queues so the last
                    # accumulators drain over parallel channels before kernel end
                    if dt < DT - 1:
                        nc.gpsimd.dma_start(out[m * P : (m + 1) * P, dsl], stage)
                    else:
                        splits = 4 if m >= MT - 2 else (2 if m >= MT - 4 else 1)
                        engs = [nc.gpsimd, nc.sync, nc.scalar]
                        rw = P // splits
                        for s in range(splits):
                            engs[s % len(engs)].dma_start(
                                out[m * P + s * rw : m * P + (s + 1) * rw, dsl],
                                stage[s * rw : (s + 1) * rw, :],
                            )
    nc.finalize()
    return nc


def _get_nc():
    if "nc" not in _CACHE:
        _CACHE["nc"] = _build_nc()
    return _CACHE["nc"]


def _prepare_in_maps(inputs):
    import ml_dtypes

    bf = ml_dtypes.bfloat16
    f32 = np.float32
    input_ = np.asarray(inputs["input"], dtype=f32)
    weight = np.asarray(inputs["weight"], dtype=f32)
    bias = np.asarray(inputs["bias"], dtype=f32)
    coef = np.asarray(inputs["coef"], dtype=f32)

    wt = np.ascontiguousarray(weight.transpose(2, 1, 0)).reshape(K, D).astype(bf)
    coefT = np.ascontiguousarray(coef.T).astype(bf)
    biasT = np.ascontiguousarray(bias.T).astype(bf)
    biasnd1 = np.ascontiguousarray((coef @ bias.T)[:, DTILE:]).astype(bf)

    coefT_f32 = coef.T  # [R, N]
    in_maps = []
    for b in range(B):
        xt = input_[b].T  # [C, N]
        xp = (coefT_f32[:, None, :] * xt[None, :, :]).reshape(K, N).astype(bf)
        in_maps.append(
            {
                "xp": np.ascontiguousarray(xp),
                "wt": wt,
                "coefT": coefT,
                "biasT": biasT,
                "biasnd1": biasnd1,
            }
        )
    return in_maps


def _install_ntff_hook_shim():
    """The agent image lacks antenv.axon_hooks; recreate it from the ctypes
    hook factory in trn_agent_boot so trace=True can capture NTFF profiles."""
    import types

    if "antenv.axon_hooks" in sys.modules:
        return
    try:
        from trn_agent_boot.trn_boot import _ntff_profile_via_ctypes

        hook = _ntff_profile_via_ctypes("/opt/axon/libaxon_pjrt.so")
        mod = types.ModuleType("antenv.axon_hooks")
        mod.get_axon_ntff_profile_hook = lambda: hook
        sys.modules["antenv.axon_hooks"] = mod
    except Exception as e:  # profiling is best-effort; execution still works
        print(f"ntff hook shim unavailable: {e}")


def _run(inputs, trace=False, **kwargs):
    from concourse.bass_utils import run_bass_kernel_spmd

    if trace:
        _install_ntff_hook_shim()
    in_maps = _prepare_in_maps(inputs)
    nc = _get_nc()
    res = run_bass_kernel_spmd(
        nc, in_maps, core_ids=list(range(N_CORES)), trace=trace, **kwargs
    )
    out = np.stack([r["out"] for r in res.results], axis=0)
    return out, res


def kernel(**inputs) -> np.ndarray:
    out, _ = _run(inputs)
    return out


# revision 4
# speedup vs baseline: 1.0766x; 1.0766x over previous
"""Trainium2 Bass kernel for nn_MixtureLinear.

Math:  out[b,n,d] = sum_{c,r} input[b,n,c] * weight[d,c,r] * coef[n,r]
                    + sum_r coef[n,r] * bias[d,r]

Sharding: data-parallel over batch (B == 8 == n_cores).

Per-core formulation: ONE fused matmul with contraction K = C*R = 8192 by
folding coef into the activations host-side:
    xp[r*C+c, n] = input[b,n,c] * coef[n,r]      (lhsT, resident in SBUF)
    wt[r*C+c, d] = weight[d,c,r]                 (rhs, streamed from HBM)
    out[n, d]    = xp.T @ wt + coef @ bias.T
Each [128n x 512d] output tile accumulates the full contraction in one PSUM
bank with no intermediate drains. The last J k-tiles run as fp8-e4m3
DoubleRow matmuls (2 k-planes per instruction, ~2x MAC rate; measured
224.8ns for K=256/F=512 vs 216ns for bf16 K=128/F=512) with
product-preserving scaling xp/8, wt*8 — exact-data numpy study: rel err
0.0138 at J=4 vs the 2e-2 gate.

Bias: first d-half via K=8 matmuls that double as PE-rate warmup ramp
fodder; second half added by the DVE during the PSUM drain.

Schedule notes (from perfetto traces):
- ~6us fixed preamble before queues run; first DMA transfers ~2us after
  issue; each dma_start costs ~0.6us of ISSUE time on its queue, so the
  three DMA-capable queues (sync/scalar/gpsimd) are loaded in parallel.
- F=64 dummy matmuls on a memset tile start the PE p-state ramp (~3us to
  full speed) while the first real tiles stream in.
- gpsimd's end-of-queue DRAIN costs ~8us; its last issue is mid-kernel
  (dt0 stores) so the drain overlaps compute instead of the exit barrier.
- dt1 closes each output tile early (m-major tail over the last k-tiles)
  so drains + stores pipeline against the remaining matmuls.
"""

import sys

if "/opt/trn_rl_repo" not in sys.path:
    sys.path.insert(0, "/opt/trn_rl_repo")

import numpy as np

B, N, C, D, R = 8, 1024, 1024, 1024, 8
P = 128        # SBUF partitions
DTILE = 512    # matmul moving free dim (one fp32 PSUM bank)
K = C * R      # fused contraction
KT = K // P    # 64 contraction tiles
MT = N // P    # 8 token tiles
DT = D // DTILE  # 2 output column tiles
N_CORES = 8
NDUMMY = 20    # warmup matmuls (F=64) to ramp the PE p-state during DMA wait
J = 4          # k-tiles (of KT) computed in fp8 DoubleRow; even, >= 0
KB = KT - J    # bf16 k-tiles
JD = J // 2    # DoubleRow instructions per (m, dt)
MTAIL = 2      # bf16 k-rows folded into dt1's m-major tail

_CACHE = {}


def _build_nc():
    import concourse.mybir as mybir
    import concourse.tile as tile
    from concourse import bacc

    f32 = mybir.dt.float32
    bf16 = mybir.dt.bfloat16
    fp8 = mybir.dt.float8e4
    mult = mybir.AluOpType.mult
    add = mybir.AluOpType.add
    DR = mybir.MatmulPerfMode.DoubleRow

    nc = bacc.Bacc()
    xp = nc.dram_tensor("xp", [KB * P, N], bf16, kind="ExternalInput")
    wt = nc.dram_tensor("wt", [KB * P, D], bf16, kind="ExternalInput")
    coefT = nc.dram_tensor("coefT", [R, N], bf16, kind="ExternalInput")
    biasT = nc.dram_tensor("biasT", [R, D], bf16, kind="ExternalInput")
    biasnd1 = nc.dram_tensor("biasnd1", [N, DTILE], bf16, kind="ExternalInput")
    out = nc.dram_tensor("out", [N, D], f32, kind="ExternalOutput")
    if J:
        xp8 = nc.dram_tensor("xp8", [JD * P, 2 * N], fp8, kind="ExternalInput")
        wt8 = nc.dram_tensor("wt8", [DT * JD * P, 2 * DTILE], fp8, kind="ExternalInput")

    with tile.TileContext(nc) as tc:
        with (
            tc.tile_pool(name="consts", bufs=1) as cpool,
            tc.tile_pool(name="wpool", bufs=22) as wpool,
            tc.tile_pool(name="w8pool", bufs=4) as w8pool,
            tc.tile_pool(name="stpool", bufs=6) as stpool,
            tc.tile_pool(name="psum", bufs=1, space="PSUM") as pspool,
        ):
            ps = [
                pspool.tile([P, DTILE], f32, name=f"ps{m}", tag=f"ps{m}", bufs=1)
                for m in range(MT)
            ]

            # warmup: PE ramp fodder with no DMA dependency
            warm = cpool.tile([P, 64], bf16, name="warm", tag="warm")
            nc.vector.memset(warm, 0.0)
            for _ in range(NDUMMY):
                nc.tensor.matmul(
                    ps[0][0:64, 0:64], warm, warm[:, 0:64], start=True, stop=True
                )

            # --- DMA issue streams (issue cost ~0.6us each; 3 queues) ---
            # scalar: tiny bias operands first (gate the bias warmup matmuls)
            coefT_sb = cpool.tile([R, N], bf16, name="coefT_sb", tag="coefT_sb")
            nc.scalar.dma_start(coefT_sb, coefT[:, :])
            biasT_sb = cpool.tile([R, D], bf16, name="biasT_sb", tag="biasT_sb")
            nc.scalar.dma_start(biasT_sb, biasT[:, :])

            # sync: the whole wt stream in consumption order
            wt_sb = {}
            for dt in range(DT):
                for k in range(KB):
                    w = wpool.tile([P, DTILE], bf16, name="w", tag="w")
                    nc.sync.dma_start(
                        w, wt[k * P : (k + 1) * P, dt * DTILE : (dt + 1) * DTILE]
                    )
                    wt_sb[k, dt] = w
                for kk in range(JD):
                    w = w8pool.tile([P, 2, DTILE], fp8, name="w8", tag="w8")
                    nc.sync.dma_start(
                        w, wt8[(dt * JD + kk) * P : (dt * JD + kk + 1) * P, :]
                    )
                    wt_sb[KB + kk, dt] = w

            # gpsimd: xp stream (first tile split so matmuls can start early)
            xp_sb = [
                cpool.tile([P, N], bf16, name=f"xp{k}", tag=f"xp{k}")
                for k in range(KB)
            ]
            nc.gpsimd.dma_start(xp_sb[0][:, 0:512], xp[0:P, 0:512])
            nc.gpsimd.dma_start(xp_sb[0][:, 512:1024], xp[0:P, 512:1024])
            for k in range(1, KB):
                nc.gpsimd.dma_start(xp_sb[k], xp[k * P : (k + 1) * P, :])
            xp8_sb = []
            for kk in range(JD):
                t = cpool.tile([P, 2, N], fp8, name=f"xp8_{kk}", tag=f"xp8_{kk}")
                nc.gpsimd.dma_start(t, xp8[kk * P : (kk + 1) * P, :])
                xp8_sb.append(t)

            def mm(m, k, dt, start, stop):
                if k < KB:
                    nc.tensor.matmul(
                        ps[m],
                        xp_sb[k][:, m * P : (m + 1) * P],
                        wt_sb[k, dt],
                        start=start,
                        stop=stop,
                    )
                else:
                    nc.tensor.matmul(
                        ps[m],
                        xp8_sb[k - KB][:, :, m * P : (m + 1) * P],
                        wt_sb[k, dt],
                        start=start,
                        stop=stop,
                        perf_mode=DR,
                    )

            NK = KB + JD  # instructions per (m, dt) chain, minus bias
            for dt in range(DT):
                dsl = slice(dt * DTILE, (dt + 1) * DTILE)
                if dt == 0:
                    # bias via K=8 matmul: initializes the accumulation group
                    # and doubles as (productive) ramp work
                    for m in range(MT):
                        nc.tensor.matmul(
                            ps[m],
                            coefT_sb[:, m * P : (m + 1) * P],
                            biasT_sb[:, dsl],
                            start=True,
                            stop=False,
                        )
                    for k in range(NK):
                        for m in range(MT):
                            mm(m, k, dt, False, k == NK - 1)
                    # drains split across scalar+vector so the banks free in
                    # time for dt1's start=True matmuls
                    for m in range(MT):
                        stage = stpool.tile([P, DTILE], f32, name="st", tag="st")
                        if m % 2 == 0:
                            nc.scalar.copy(stage, ps[m])
                        else:
                            nc.vector.tensor_scalar(stage, ps[m], 1.0, None, mult)
                        nc.gpsimd.dma_start(out[m * P : (m + 1) * P, dsl], stage)
                    # biasnd for dt1's drains (scalar queue, now idle)
                    bias1_sb = []
                    for m in range(MT):
                        t = cpool.tile(
                            [P, DTILE], bf16, name=f"b1_{m}", tag=f"b1_{m}"
                        )
                        nc.scalar.dma_start(t, biasnd1[m * P : (m + 1) * P, :])
                        bias1_sb.append(t)
                else:
                    for k in range(NK - MTAIL - JD):
                        for m in range(MT):
                            mm(m, k, dt, k == 0, False)
                    # m-major tail: close each bank early so drain+store
                    # pipelines against the remaining matmuls
                    for m in range(MT):
                        for k in range(NK - MTAIL - JD, NK):
                            mm(m, k, dt, False, k == NK - 1)
                        stage = stpool.tile([P, DTILE], f32, name="st", tag="st")
                        nc.vector.scalar_tensor_tensor(
                            stage, ps[m], 1.0, bias1_sb[m], mult, add
                        )
                        splits = 2 if m >= MT - 2 else 1
                        engs = [nc.sync, nc.scalar]
                        rw = P // splits
                        for s in range(splits):
                            engs[(m + s) % 2].dma_start(
                                out[m * P + s * rw : m * P + (s + 1) * rw, dsl],
                                stage[s * rw : (s + 1) * rw, :],
                            )
    nc.finalize()
    return nc


def _get_nc():
    if "nc" not in _CACHE:
        _CACHE["nc"] = _build_nc()
    return _CACHE["nc"]


def _prepare_in_maps(inputs):
    import ml_dtypes

    bf = ml_dtypes.bfloat16
    f8 = ml_dtypes.float8_e4m3fn
    f32 = np.float32
    input_ = np.asarray(inputs["input"], dtype=f32)
    weight = np.asarray(inputs["weight"], dtype=f32)
    bias = np.asarray(inputs["bias"], dtype=f32)
    coef = np.asarray(inputs["coef"], dtype=f32)

    wt_full = np.ascontiguousarray(weight.transpose(2, 1, 0)).reshape(K, D)
    wt = np.ascontiguousarray(wt_full[: KB * P]).astype(bf)
    coefT = np.ascontiguousarray(coef.T).astype(bf)
    biasT = np.ascontiguousarray(bias.T).astype(bf)
    biasnd1 = np.ascontiguousarray((coef @ bias.T)[:, DTILE:]).astype(bf)

    shared = {"wt": wt, "coefT": coefT, "biasT": biasT, "biasnd1": biasnd1}
    if J:
        w8 = (wt_full[KB * P :] * 8.0).astype(f8)  # [J*P, D]
        # [dt, kk, p, i, f] -> rows (dt*JD+kk)*P+p, cols i*DTILE+f
        w8r = w8.reshape(JD, 2, P, DT, DTILE)
        shared["wt8"] = np.ascontiguousarray(
            w8r.transpose(3, 0, 2, 1, 4).reshape(DT * JD * P, 2 * DTILE)
        )

    coefT_f32 = coef.T  # [R, N]
    in_maps = []
    for b in range(B):
        xt = input_[b].T  # [C, N]
        xpf = (coefT_f32[:, None, :] * xt[None, :, :]).reshape(K, N)
        m = {"xp": np.ascontiguousarray(xpf[: KB * P]).astype(bf), **shared}
        if J:
            x8 = (xpf[KB * P :] / 8.0).astype(f8)  # [J*P, N]
            m["xp8"] = np.ascontiguousarray(
                x8.reshape(JD, 2, P, N).transpose(0, 2, 1, 3).reshape(JD * P, 2 * N)
            )
        in_maps.append(m)
    return in_maps


def _install_ntff_hook_shim():
    """The agent image lacks antenv.axon_hooks; recreate it from the ctypes
    hook factory in trn_agent_boot so trace=True can capture NTFF profiles."""
    import types

    if "antenv.axon_hooks" in sys.modules:
        return
    try:
        from trn_agent_boot.trn_boot import _ntff_profile_via_ctypes

        hook = _ntff_profile_via_ctypes("/opt/axon/libaxon_pjrt.so")
        mod = types.ModuleType("antenv.axon_hooks")
        mod.get_axon_ntff_profile_hook = lambda: hook
        sys.modules["antenv.axon_hooks"] = mod
    except Exception as e:  # profiling is best-effort; execution still works
        print(f"ntff hook shim unavailable: {e}")


def _run(inputs, trace=False, **kwargs):
    from concourse.bass_utils import run_bass_kernel_spmd

    if trace:
        _install_ntff_hook_shim()
    in_maps = _prepare_in_maps(inputs)
    nc = _get_nc()
    res = run_bass_kernel_spmd(
        nc, in_maps, core_ids=list(range(N_CORES)), trace=trace, **kwargs
    )
    out = np.stack([r["out"] for r in res.results], axis=0)
    return out, res


def kernel(**inputs) -> np.ndarray:
    out, _ = _run(inputs)
    return out
